# revision 24
# baseline (speedup 1.0000x reference)
"""Trainium2 Bass kernel: MultiHeadSelfAttention (B=2, S=2048, D=1024, H=16).

Self-contained. Accepts FULL inputs, returns FULL output.

Sharding (8 cores, SPMD, no collectives):
  core c -> batch b = c // 4, lane j = c % 4. Within a batch the 16 heads
  are sorted by valid_len (desc) and dealt round-robin to the 4 lanes, so
  slot i on every core holds a head from rank-quartet i. Each core computes
  q/k/v projections for its 4 heads, attention, and the row-parallel
  partial of the output projection (A @ Wo[:, heads].T, shape (S, D)).
  Host sums the 4 partials per batch.

The program is specialized to per-slot QUERY WIDTHS: width[i] =
ceil(max valid_len in rank-quartet i / 64) * 64. Query rows beyond a
slot's width are entirely masked rows, whose attention output is exactly
uniform (= mean of V), so they are filled from a precomputed mean-V column
instead of being computed. Scores/exp/attn@V/norm for the last (partial)
512-chunk of a slot run only over the first W<=512 columns -- engine cost
is linear in the free-dim width, so this prunes real work. One program
serves all 8 cores; distinct width tuples compile separately and cache.

Pipeline: one flat stream of (site, key-tile-pair) steps. Per step the PE
emits a packed score matmul pair, ACT the exp; attn@V drains L=12 steps
behind (deep ex ring), so exp never waits on V availability and attn@V
never waits on exp. Projections (k/q/v), mean-V, fills and per-token-tile
output-projection units are deadline-scheduled fillers between steps.

Device-side math notes:
  - All matmuls run in bf16 (fp32 PSUM accumulation). Score matmuls for
    two consecutive key-tiles run CONCURRENTLY on disjoint PE row-groups
    (the head's 64 k/q dims are duplicated into both partition halves;
    measured: the second matmul of each pair costs ~3ns).
  - The reference masks ENTIRE query rows j >= valid_len to -1e6 before
    softmax, making those rows' attention exactly uniform (1/S each). For
    masked rows inside a computed chunk we multiply q by the row mask:
    masked query -> scores all 0 -> exp all 1 -> uniform attention.
  - No max-subtraction in softmax: scores/8 are bounded (|s| < ~10), so
    exp() cannot overflow in fp32 and softmax is scale-invariant anyway.
  - Softmax denominator comes free from a ones-column appended to V
    (attn@V_aug yields sum(exp) in the extra output row).
  - Normalization is DMA-free: DVE reciprocal of the denominator row,
    bf16 round, PE broadcast across 64 partitions via a ones[1,64]
    stationary matmul, DVE multiply. (bf16 reciprocal adds <0.2% relative
    error; masked rows stay exact since 1/2048 is a power of two.)
  - bq/bk/bv are zeros in this problem's setup_inputs. bv/bo are folded in
    EXACTLY on the host (rows of attn sum to 1, so attn@(v+bv) = attn@v+bv).
    If bq/bk were ever nonzero we fall back to a numpy reference path.
"""

import numpy as np

B, S, D = 2, 2048, 1024
H, DH = 16, 64
HPG = 4                 # heads per core
GW = HPG * DH           # 256
P = 128
N_CORES = 8
NCH = S // 512          # query chunks
AV_LAG = 16             # attn@V drains this many steps behind scores
EX_BUFS = 18            # ex ring depth (> AV_LAG)

_PROGS = {}             # widths tuple -> compiled Bacc


def _to_bf16(a):
    import ml_dtypes
    return np.ascontiguousarray(np.asarray(a, dtype=np.float32)
                                .astype(ml_dtypes.bfloat16))


def _emit(tc, aps, widths):
    """Emit the per-core program. widths: 4 per-slot query widths (64k)."""
    from collections import deque
    from contextlib import ExitStack

    import concourse.mybir as mybir

    nc = tc.nc
    f32 = mybir.dt.float32
    bf16 = mybir.dt.bfloat16
    EXP = mybir.ActivationFunctionType.Exp
    COPY = mybir.ActivationFunctionType.Copy
    LN = mybir.ActivationFunctionType.Ln

    xT, wqT, wkT, wvT, woT, mask, out = (
        aps["xT"], aps["wqT"], aps["wkT"], aps["wvT"], aps["woT"],
        aps["mask"], aps["out"],
    )
    nchunks = [(w + 511) // 512 for w in widths]  # computed chunks per slot

    def swidth(slot, i4):
        """Width of slot's chunk i4 (<=512, multiple of 64)."""
        return min(512, widths[slot] - 512 * i4)

    pw = [max(widths[0], widths[1]), max(widths[2], widths[3])]
    pchunks = [(w + 511) // 512 for w in pw]      # q chunks per pair

    ctx = ExitStack()
    with ctx:
        sb = ctx.enter_context(tc.tile_pool(name="sb", bufs=1))
        # PSUM banks: scores 2x2 + projections 2x1 + attn@V accum 2x1 = 8
        ps_s = ctx.enter_context(tc.tile_pool(name="ps_s", bufs=2,
                                              space="PSUM"))
        ps_p = ctx.enter_context(tc.tile_pool(name="ps_p", bufs=2,
                                              space="PSUM"))
        psav = ctx.enter_context(tc.tile_pool(name="psav", bufs=2,
                                              space="PSUM"))
        rot = ctx.enter_context(tc.tile_pool(name="rot", bufs=EX_BUFS))
        ost = ctx.enter_context(tc.tile_pool(name="ost", bufs=3))
        sml = ctx.enter_context(tc.tile_pool(name="sml", bufs=4))
        xw = ctx.enter_context(tc.tile_pool(name="xw", bufs=1))
        qd_pool = ctx.enter_context(tc.tile_pool(name="qd", bufs=1))

        # persistent intermediates
        q_sb = [sb.tile([P, S], bf16, name=f"q{p}") for p in range(2)]
        k_sb = [sb.tile([P, S], bf16, name=f"k{p}") for p in range(2)]
        v_sb = [sb.tile([P, HPG, DH + 1], bf16, name=f"v{t}")
                for t in range(16)]
        a_sb = [sb.tile([P, S], bf16, name=f"a{c}") for c in range(2)]
        meanv = sb.tile([64, HPG], bf16, name="meanv")
        ones_sb = sb.tile([1, 64], bf16, name="ones")
        # per-head k with the head's 64 dims duplicated into both partition
        # halves: lets two key-tiles' score matmuls run CONCURRENTLY on
        # disjoint PE row-groups (tile_position packing)
        khd = [sb.tile([P, S], bf16, name=f"khd{h}") for h in range(HPG)]

        # ---- input loads: spread across the three DMA-capable queues.
        # x / mask / wo are CHUNKED column-wise into separate tiles so the
        # 512KB loads split into parallel 128KB transfers on distinct
        # hardware DMA queues AND the first k/q projections start after
        # only chunk 0 has landed (a single 512KB transfer occupies one
        # ~21GB/s queue for ~24us -- that was the entire ramp).
        # input loads split across the gpsimd+scalar rings; the sync
        # ring carries only the khd/qd duplication copies so they never
        # queue behind megabytes of input
        engs = [nc.gpsimd, nc.scalar]
        x_sb = [[xw.tile([P, 512], bf16, name=f"x{d}_{c}")
                 for c in range(NCH)] for d in range(8)]
        wq_sb = [xw.tile([P, GW], bf16, name=f"wq{d}") for d in range(8)]
        wk_sb = [xw.tile([P, GW], bf16, name=f"wk{d}") for d in range(8)]
        wv_sb = [xw.tile([P, GW], bf16, name=f"wv{d}") for d in range(8)]
        mk_sb = [[xw.tile([P, 512], bf16, name=f"mk{p}_{c}")
                  for c in range(NCH)] for p in range(2)]
        wo_sb2 = [[xw.tile([P, 512], bf16, name=f"wo{c}_{n}")
                   for n in range(2)] for c in range(2)]
        # warm the ACT exp table-set (~2.7us load) during the DMA phase
        # so the first real exp doesn't pay it on the critical path
        warm_in = sml.tile([1, 8], f32, name="warm_in")
        warm_out = sml.tile([1, 8], f32, name="warm_out")
        nc.any.memset(warm_in[:], 0.0)
        nc.scalar.activation(warm_out[:], warm_in[:], EXP,
                             bias=0.0, scale=0.125)
        nc.any.memset(ones_sb[:], 1.0)

        # prologue loads ONLY what the first k/q chunk needs; every
        # later input is a deadline filler so the engine rings stay short
        # (a dma_start waits behind all earlier ones on its ring)
        dma_n = [0]

        def dma(dst, src_ap):
            engs[dma_n[0] % 2].dma_start(dst, src_ap)
            dma_n[0] += 1

        for d in range(8):
            dma(wk_sb[d][:], wkT[d * P:(d + 1) * P, :])
        for d in range(8):
            dma(wq_sb[d][:], wqT[d * P:(d + 1) * P, :])
        for d in range(8):
            dma(x_sb[d][0][:], xT[0, d * P:(d + 1) * P, :])
        dma(mk_sb[0][0][:], mask[0, 0])

        def load_x_chunk(c):
            for d in range(8):
                dma(x_sb[d][c][:], xT[c, d * P:(d + 1) * P, :])
            dma(mk_sb[0][c][:], mask[0, c])

        def load_wv():
            for d in range(8):
                dma(wv_sb[d][:], wvT[d * P:(d + 1) * P, :])

        def load_mask1():
            for c in range(NCH):
                dma(mk_sb[1][c][:], mask[1, c])

        def load_wo():
            for c in range(2):
                for n2 in range(2):
                    dma(wo_sb2[c][n2][:], woT[c, n2, :, :])

        # ---- emitters ----------------------------------------------------
        k_pt = {}

        def emit_k_half(mt, n4, half):
            if half == 0:
                k_pt[(mt, n4)] = ps_p.tile([P, 512], f32, name="ps_p")
            pt = k_pt[(mt, n4)]
            for d in range(4 * half, 4 * half + 4):
                nc.tensor.matmul(
                    pt[:],
                    wk_sb[d][:, mt * P:(mt + 1) * P],
                    x_sb[d][n4][:],
                    start=(d == 0), stop=(d == 7),
                )
            if half == 0:
                return
            del k_pt[(mt, n4)]
            nc.vector.tensor_copy(
                k_sb[mt][:, n4 * 512:(n4 + 1) * 512], pt[:])
            cs = slice(n4 * 512, (n4 + 1) * 512)
            for rr in range(2):
                h = 2 * mt + rr
                src_ap = k_sb[mt][64 * rr:64 * rr + 64, cs]
                nc.sync.dma_start(khd[h][0:64, cs], src_ap)
                nc.sync.dma_start(khd[h][64:128, cs], src_ap)

        def emit_k_chunk(mt, n4):
            emit_k_half(mt, n4, 0)
            emit_k_half(mt, n4, 1)

        q_pt = {}

        def emit_q_half(mt, n4, half):
            qw = min(512, pw[mt] - 512 * n4)
            if half == 0:
                q_pt[(mt, n4)] = ps_p.tile([P, 512], f32, name="ps_p")
            pt = q_pt[(mt, n4)]
            for d in range(4 * half, 4 * half + 4):
                nc.tensor.matmul(
                    pt[:, :qw],
                    wq_sb[d][:, mt * P:(mt + 1) * P],
                    x_sb[d][n4][:, :qw],
                    start=(d == 0), stop=(d == 7),
                )
            if half == 0:
                return
            del q_pt[(mt, n4)]
            cs = slice(n4 * 512, n4 * 512 + qw)
            # fold the row mask into q (masked query -> q = 0)
            nc.vector.tensor_mul(
                q_sb[mt][:, cs], pt[:, :qw], mk_sb[mt][n4][:, :qw])
            for rr in range(2):
                if n4 >= nchunks[2 * mt + rr]:
                    continue
                sw = swidth(2 * mt + rr, n4)
                qd = qd_pool.tile([P, 512], bf16, name=f"qd{mt}_{n4}_{rr}")
                qd_tiles[(mt, n4, rr)] = qd
                src_ap = q_sb[mt][64 * rr:64 * rr + 64,
                                  n4 * 512:n4 * 512 + sw]
                nc.sync.dma_start(qd[0:64, :sw], src_ap)
                nc.sync.dma_start(qd[64:128, :sw], src_ap)

        def emit_q_chunk(mt, n4):
            emit_q_half(mt, n4, 0)
            emit_q_half(mt, n4, 1)

        def emit_v_tile(t):
            pt = ps_p.tile([P, 512], f32, name="ps_p")[:, :GW]
            for d in range(8):
                nc.tensor.matmul(
                    pt,
                    x_sb[d][t // 4][:, (t % 4) * P:(t % 4 + 1) * P],
                    wv_sb[d][:],
                    start=(d == 0), stop=(d == 7),
                )
            nc.any.memset(v_sb[t][:], 1.0)   # ones column at [:, :, DH]
            nc.vector.tensor_copy(
                v_sb[t][:, :, 0:DH],
                pt.rearrange("p (h e) -> p h e", h=HPG))

        def emit_meanv():
            # mean of V per head (= output of fully-masked query rows)
            pmv = ps_p.tile([P, 512], f32, name="ps_p")[:DH + 1, :HPG]
            for h in range(HPG):
                for jt in range(16):
                    nc.tensor.matmul(
                        pmv[:, h:h + 1],
                        v_sb[jt][:, h, :],
                        v_sb[jt][:, h, DH:DH + 1],  # the ones column
                        start=(jt == 0), stop=(jt == 15),
                    )
            nc.scalar.activation(meanv[:], pmv[:DH, :], COPY,
                                 bias=0.0, scale=1.0 / S)

        def emit_fills():
            # masked query cols: attention output is exactly mean-of-V
            for pair in range(2):
                for rr in range(2):
                    h = 2 * pair + rr
                    rows = slice(64 * rr, 64 * rr + 64)
                    if widths[h] < S:
                        nc.vector.tensor_copy(
                            a_sb[pair][rows, widths[h]:S],
                            meanv[:, h:h + 1].to_broadcast(
                                (64, S - widths[h])),
                        )

        def emit_final_unit(i4, t4, tail):
            """Output projection for one token tile."""
            t = i4 * 4 + t4
            ot = ost.tile([P, D], f32, name="ot")
            for n2 in range(2):
                pf = ps_p.tile([P, 512], f32, name="ps_p")
                for c in range(2):
                    nc.tensor.matmul(
                        pf[:],
                        a_sb[c][:, t * P:(t + 1) * P],
                        wo_sb2[c][n2][:],
                        start=(c == 0), stop=(c == 1),
                    )
                if tail and n2 == 1:
                    # exp stream is drying up: borrow the ACT for half
                    # the copies so the ps_p rotation keeps pace
                    nc.scalar.activation(
                        ot[:, n2 * 512:(n2 + 1) * 512], pf[:], COPY)
                else:
                    nc.vector.tensor_copy(
                        ot[:, n2 * 512:(n2 + 1) * 512], pf[:])
            (nc.scalar if t % 2 else nc.gpsimd).dma_start(
                out[t * P:(t + 1) * P, :], ot[:])

        class Site:
            """One (chunk, pair, head-row) attention block."""

            def __init__(self, i4, pair, rr):
                self.i4, self.pair, self.rr = i4, pair, rr
                self.h = 2 * pair + rr
                self.W = swidth(self.h, i4)
                self.rows = slice(64 * rr, 64 * rr + 64)
                self.qs = slice(i4 * 512, i4 * 512 + self.W)
                self.pav = psav.tile([DH + 1, 512], f32, name="psav")
                self.pses = []
                self.exs = []

            def emit_scores(self, jtp):
                W = self.W
                pse = ps_s.tile([P, 2, 512], f32, name="ps_s")
                # the two key-tiles use disjoint PE row-groups (partitions
                # 0-63 / 64-127 of the duplicated khd/qd tiles), so they
                # execute concurrently (the 2nd matmul costs ~3ns)
                qd = qd_tiles[(self.pair, self.i4, self.rr)]
                for jj in range(2):
                    jt = jtp * 2 + jj
                    half = slice(64 * jj, 64 * jj + 64)
                    # scores^T = k @ q^T for head h
                    nc.tensor.matmul(
                        pse[:, jj, :W],
                        khd[self.h][half, jt * P:(jt + 1) * P],
                        qd[half, :W],
                        start=True, stop=True,
                    )
                self.pses.append(pse)

            def emit_exp(self, jtp):
                W = self.W
                ex = rot.tile([P, 2, 512], bf16, name="ex")
                nc.scalar.activation(ex[:, :, :W], self.pses[jtp][:, :, :W],
                                     EXP, bias=0.0, scale=0.125)
                self.exs.append(ex)

            def emit_av(self, jtp):
                W = self.W
                ex = self.exs[jtp]
                for jj in range(2):
                    jt = jtp * 2 + jj
                    nc.tensor.matmul(
                        self.pav[:, :W],
                        v_sb[jt][:, self.h, :],
                        ex[:, jj, :W],
                        start=(jtp == 0 and jj == 0),
                        stop=(jtp == 7 and jj == 1),
                    )

            def emit_norm_a(self):
                # DMA-free normalization, phase A (DVE): exact reciprocal()
                # on one partition costs 3.3us (it iterates per element on
                # a single lane); the approx custom op (~51 ULP, far below
                # the bf16 rounding below) is ~5x faster. It reads the
                # denominator via a staging copy: fed straight from the
                # psum row it returns garbage (microtested fine from
                # SBUF/PSUM at offset 0, so it is the offset-64 psum row
                # of an accumulating tile it mishandles).
                W = self.W
                self.rcp = sml.tile([1, 512], bf16, name="rcp")
                dn = sml.tile([1, 512], f32, name="dn")
                nc.vector.tensor_copy(dn[:, :W], self.pav[DH:DH + 1, :W])
                r32 = sml.tile([1, 512], f32, name="r32")
                nc.vector.reciprocal_approx_fast(
                    out=r32[:, :W], in_=dn[:, :W])
                nc.vector.tensor_copy(self.rcp[:, :W], r32[:, :W])

            def emit_norm_b(self):
                # phase B (two steps later, so the PE never waits on phase
                # A): broadcast across 64 partitions via a ones[1,64]
                # stationary matmul, bounce to SBUF (exact: values are
                # already bf16-rounded; DVE reads only one PSUM operand),
                # multiply into a_sb.
                W = self.W
                rcb = ps_p.tile([P, 512], f32, name="ps_p")
                nc.tensor.matmul(rcb[:64, :W], ones_sb[:],
                                 self.rcp[:, :W], start=True, stop=True)
                rcs = sml.tile([64, 512], bf16, name="rcs")
                nc.vector.tensor_copy(rcs[:, :W], rcb[:64, :W])
                nc.vector.tensor_mul(
                    a_sb[self.pair][self.rows, self.qs],
                    self.pav[0:DH, :W], rcs[:, :W])

        # ---- stream schedule --------------------------------------------
        # pair 0 leads, pair 1 lags one chunk
        site_list = []
        for i4 in range(NCH + 1):
            if i4 < NCH:
                for rr in range(2):
                    if i4 < nchunks[rr]:
                        site_list.append((i4, 0, rr))
            if 1 <= i4:
                for rr in range(2):
                    if i4 - 1 < nchunks[2 + rr]:
                        site_list.append((i4 - 1, 1, rr))

        first_step = {}        # (i4, pair) -> first step of any matching site
        for j, (i4, pair, rr) in enumerate(site_list):
            first_step.setdefault((i4, pair), 8 * j)
        fs1 = min((8 * j for j, s in enumerate(site_list) if s[1] == 1),
                  default=10**9)

        qd_tiles = {}

        # deadline-scheduled fillers: (due_step, order, fn)
        fillers = []
        fillers.append((0, -1, lambda: load_x_chunk(1)))
        fillers.append((1, -1, load_wv))
        fillers.append((2, -1, lambda: load_x_chunk(2)))
        fillers.append((4, -1, lambda: load_x_chunk(3)))
        # mask[1] MUST be emitted before the first q(pair1) filler: a
        # read emitted before its producing DMA gets no dependency at all
        q1_due = max(1, first_step[(0, 1)] - 10)
        fillers.append((max(1, q1_due - 2), -1, load_mask1))
        fillers.append((24, -1, load_wo))
        for c in range(1, NCH):
            for half in range(2):
                fillers.append((2 * c - 2 + half, 0,
                                lambda c=c, h=half: emit_k_half(0, c, h)))
        for c in range(NCH):
            for half in range(2):
                fillers.append((max(8, fs1 - 14 + 3 * c + half), 1,
                                lambda c=c, h=half: emit_k_half(1, c, h)))
        for pair in range(2):
            for n4 in range(pchunks[pair]):
                if (pair, n4) == (0, 0):
                    continue
                due = max(1, first_step[(n4, pair)] - 10)
                for half in range(2):
                    fillers.append((due + half, 2,
                                    lambda p=pair, n=n4, h=half:
                                    emit_q_half(p, n, h)))
        for t in range(16):
            fillers.append((4 + t, 3, lambda t=t: emit_v_tile(t)))
        fillers.append((21, 4, emit_meanv))
        fillers.append((21, 5, emit_fills))
        fillers.sort(key=lambda f: (f[0], f[1]))
        fptr = [0]

        def do_fillers(step):
            while fptr[0] < len(fillers) and fillers[fptr[0]][0] <= step:
                fillers[fptr[0]][2]()
                fptr[0] += 1

        norms_left = [0] * NCH
        for (i4, pair, rr) in site_list:
            norms_left[i4] += 1
        pending_finals = deque()
        n_units = [0]
        total_units = 4 * NCH

        def queue_final(c):
            for t4 in range(4):
                pending_finals.append((c, t4))

        def emit_one_final():
            if pending_finals:
                c, t4 = pending_finals.popleft()
                n_units[0] += 1
                emit_final_unit(c, t4, tail=(n_units[0] > total_units - 8))

        normb_q = deque()     # (due_step, site)

        def drain(sj, step):
            site, jtp = sj
            site.emit_av(jtp)
            if jtp == 7:
                site.emit_norm_a()
                normb_q.append((step + 2, site))

        def do_normb(step):
            while normb_q and normb_q[0][0] <= step:
                site = normb_q.popleft()[1]
                site.emit_norm_b()
                norms_left[site.i4] -= 1
                if norms_left[site.i4] == 0:
                    queue_final(site.i4)

        # prologue
        emit_k_chunk(0, 0)
        emit_q_chunk(0, 0)

        avq = deque()
        step = 0
        for key in site_list:
            site = Site(*key)
            for jtp in range(8):
                do_fillers(step)
                do_normb(step)
                site.emit_scores(jtp)
                site.emit_exp(jtp)
                avq.append((site, jtp))
                if len(avq) > AV_LAG:
                    drain(avq.popleft(), step)
                emit_one_final()
                step += 1
        while avq or normb_q:
            do_fillers(step)
            do_normb(step)
            if avq:
                drain(avq.popleft(), step)
            emit_one_final()
            step += 1
        do_fillers(10**9)
        while pending_finals:
            emit_one_final()


def build_program(widths):
    """Build + schedule + compile the per-core program (cached per key)."""
    widths = tuple(widths)
    if widths in _PROGS:
        return _PROGS[widths]

    import concourse.mybir as mybir
    import concourse.tile as tile
    from concourse import bacc

    nc = bacc.Bacc("TRN2", target_bir_lowering=False, debug=False)
    f32 = mybir.dt.float32
    bf16 = mybir.dt.bfloat16
    aps = {
        "xT": nc.dram_tensor("xT", [NCH, D, 512], bf16,
                             kind="ExternalInput").ap(),
        "wqT": nc.dram_tensor("wqT", [D, GW], bf16, kind="ExternalInput").ap(),
        "wkT": nc.dram_tensor("wkT", [D, GW], bf16, kind="ExternalInput").ap(),
        "wvT": nc.dram_tensor("wvT", [D, GW], bf16, kind="ExternalInput").ap(),
        "woT": nc.dram_tensor("woT", [2, 2, P, 512], bf16,
                              kind="ExternalInput").ap(),
        "mask": nc.dram_tensor("mask", [2, NCH, P, 512], bf16,
                               kind="ExternalInput").ap(),
        "out": nc.dram_tensor("out", [S, D], f32, kind="ExternalOutput").ap(),
    }
    with tile.TileContext(nc) as tc:
        _emit(tc, aps, widths)
    nc.compile()
    _PROGS[widths] = nc
    return nc


def plan(valid_lens):
    """Head->core assignment and the compile-time width tuple.

    Returns (widths, heads_per_core): heads_per_core[c] lists the 4
    global head indices (within core c's batch) in slot order.
    """
    valid = np.asarray(valid_lens).reshape(B, H)
    heads_per_core = [None] * N_CORES
    quart_max = [0] * HPG
    for b in range(B):
        order = np.argsort(-valid[b], kind="stable")
        for j in range(HPG):
            hs = [int(order[4 * i + j]) for i in range(HPG)]
            heads_per_core[b * HPG + j] = hs
        for i in range(HPG):
            quart_max[i] = max(quart_max[i],
                               int(valid[b, order[4 * i]]))
    widths = tuple(min(-(-m // 64) * 64, S) for m in quart_max)
    return widths, heads_per_core


def make_in_maps(X, Wq, Wk, Wv, Wo, valid_lens):
    """Host-side sharding: build the 8 per-core input maps."""
    import ml_dtypes
    X = np.asarray(X, dtype=np.float32)
    valid = np.asarray(valid_lens).reshape(B, H)
    widths, heads_per_core = plan(valid_lens)
    iota = np.arange(S)
    in_maps = []
    # x chunked [NCH, D, 512] so each 128KB slab is one contiguous DMA
    xTs = [np.ascontiguousarray(
        _to_bf16(X[b].T).reshape(D, NCH, 512).transpose(1, 0, 2))
        for b in range(B)]
    Wq, Wk, Wv, Wo = (np.asarray(a, np.float32) for a in (Wq, Wk, Wv, Wo))
    for c in range(N_CORES):
        b = c // HPG
        hs = heads_per_core[c]
        rows = np.concatenate([np.arange(h * DH, (h + 1) * DH) for h in hs])
        mask = np.empty((2, P, S), dtype=ml_dtypes.bfloat16)
        for p in range(2):
            for rr in range(2):
                h = hs[2 * p + rr]
                mask[p, 64 * rr:64 * rr + 64, :] = (
                    iota < int(valid[b, h])).astype(ml_dtypes.bfloat16)[None, :]
        mask_c = np.ascontiguousarray(
            mask.reshape(2, P, NCH, 512).transpose(0, 2, 1, 3))
        woT = _to_bf16(Wo[:, rows].T)          # [GW, D]
        woT_c = np.ascontiguousarray(
            woT.reshape(2, P, 2, 512).transpose(0, 2, 1, 3))
        in_maps.append({
            "xT": xTs[b],
            "wqT": _to_bf16(Wq[rows, :].T),
            "wkT": _to_bf16(Wk[rows, :].T),
            "wvT": _to_bf16(Wv[rows, :].T),
            "woT": woT_c,
            "mask": mask_c,
        })
    return widths, in_maps


def assemble(results, Wo, bv, bo):
    """Host-side unshard: sum row-parallel partials, fold bv/bo exactly."""
    out = np.zeros((B, S, D), dtype=np.float32)
    for c in range(N_CORES):
        b = c // HPG
        out[b] += results[c]["out"]
    bias = (np.asarray(bv, np.float32) @ np.asarray(Wo, np.float32).T
            + np.asarray(bo, np.float32))
    out += bias[None, None, :]
    return out


def _numpy_fallback(X, Wq, bq, Wk, bk, Wv, bv, Wo, bo, valid_lens):
    X = np.asarray(X, np.float32)
    q = (X @ np.asarray(Wq, np.float32).T + np.asarray(bq, np.float32))
    k = (X @ np.asarray(Wk, np.float32).T + np.asarray(bk, np.float32))
    v = (X @ np.asarray(Wv, np.float32).T + np.asarray(bv, np.float32))

    def split(y):
        return (y.reshape(B, S, H, DH).transpose(0, 2, 1, 3)
                .reshape(B * H, S, DH))

    q, k, v = split(q), split(k), split(v)
    s = np.einsum("bqd,bkd->bqk", q, k) / np.sqrt(DH).astype(np.float32)
    rm = (np.arange(S)[None, :]
          < np.asarray(valid_lens).reshape(-1)[:, None])
    s = np.where(rm[:, :, None], s, -1e6)
    s = s - s.max(axis=-1, keepdims=True)
    e = np.exp(s)
    attn = e / e.sum(axis=-1, keepdims=True)
    o = np.einsum("bqk,bkd->bqd", attn, v)
    o = o.reshape(B, H, S, DH).transpose(0, 2, 1, 3).reshape(B, S, D)
    return o @ np.asarray(Wo, np.float32).T + np.asarray(bo, np.float32)


def run_cores(widths, in_maps, trace=False, **kw):
    """Run the compiled program on cores 0-7."""
    from concourse.bass_utils import run_bass_kernel_spmd

    nc = build_program(widths)
    return run_bass_kernel_spmd(nc, in_maps, list(range(N_CORES)),
                                trace=trace, **kw)


def kernel(X, Wq, bq, Wk, bk, Wv, bv, Wo, bo, valid_lens):
    if np.any(np.asarray(bq)) or np.any(np.asarray(bk)):
        # never the case for this problem's setup_inputs (zeros);
        # exact fallback kept for safety.
        return _numpy_fallback(X, Wq, bq, Wk, bk, Wv, bv, Wo, bo, valid_lens)
    widths, in_maps = make_in_maps(X, Wq, Wk, Wv, Wo, valid_lens)
    res = run_cores(widths, in_maps, trace=False)
    return assemble(res.results, Wo, bv, bo)


# revision 26
# speedup vs baseline: 1.0226x; 1.0226x over previous
"""Trainium2 Bass kernel: MultiHeadSelfAttention (B=2, S=2048, D=1024, H=16).

Self-contained. Accepts FULL inputs, returns FULL output.

Sharding (8 cores, SPMD, no collectives):
  core c -> batch b = c // 4, lane j = c % 4. Within a batch the 16 heads
  are sorted by valid_len (desc) and dealt round-robin to the 4 lanes, so
  slot i on every core holds a head from rank-quartet i. Each core computes
  q/k/v projections for its 4 heads, attention, and the row-parallel
  partial of the output projection (A @ Wo[:, heads].T, shape (S, D)).
  Host sums the 4 partials per batch.

The program is specialized to per-slot QUERY WIDTHS: width[i] =
ceil(max valid_len in rank-quartet i / 64) * 64. Query rows beyond a
slot's width are entirely masked rows, whose attention output is exactly
uniform (= mean of V), so they are filled from a precomputed mean-V column
instead of being computed. Scores/exp/attn@V/norm for the last (partial)
512-chunk of a slot run only over the first W<=512 columns -- engine cost
is linear in the free-dim width, so this prunes real work. One program
serves all 8 cores; distinct width tuples compile separately and cache.

Pipeline: one flat stream of (site, key-tile-pair) steps. Per step the PE
emits a packed score matmul pair, ACT the exp; attn@V drains L=16 steps
behind (deep ex ring), so exp never waits on V availability and attn@V
never waits on exp. Projections (k/q/v), mean-V, fills and per-token-tile
output-projection units are deadline-scheduled fillers between steps.

Device-side math notes:
  - All matmuls run in bf16 (fp32 PSUM accumulation). Score matmuls for
    two consecutive key-tiles run CONCURRENTLY on disjoint PE row-groups
    (the head's 64 k/q dims are duplicated into both partition halves;
    measured: the second matmul of each pair costs ~3ns).
  - The reference masks ENTIRE query rows j >= valid_len to -1e6 before
    softmax, making those rows' attention exactly uniform (1/S each). For
    masked rows inside a computed chunk we multiply q by the row mask:
    masked query -> scores all 0 -> exp all 1 -> uniform attention.
  - No max-subtraction in softmax: scores/8 are bounded (|s| < ~10), so
    exp() cannot overflow in fp32 and softmax is scale-invariant anyway.
  - Softmax denominator comes free from a ones-column appended to V
    (attn@V_aug yields sum(exp) in the extra output row).
  - Normalization is DMA-free: DVE reciprocal of the denominator row,
    bf16 round, PE broadcast across 64 partitions via a ones[1,64]
    stationary matmul, DVE multiply. (bf16 reciprocal adds <0.2% relative
    error; masked rows stay exact since 1/2048 is a power of two.)
  - bq/bk/bv are zeros in this problem's setup_inputs. bv/bo are folded in
    EXACTLY on the host (rows of attn sum to 1, so attn@(v+bv) = attn@v+bv).
    If bq/bk were ever nonzero we fall back to a numpy reference path.
"""

import numpy as np

B, S, D = 2, 2048, 1024
H, DH = 16, 64
HPG = 4                 # heads per core
GW = HPG * DH           # 256
P = 128
N_CORES = 8
NCH = S // 512          # query chunks
AV_LAG = 16             # attn@V drains this many steps behind scores
EX_BUFS = 18            # ex ring depth (> AV_LAG)

_PROGS = {}             # widths tuple -> compiled Bacc


def _to_bf16(a):
    import ml_dtypes
    return np.ascontiguousarray(np.asarray(a, dtype=np.float32)
                                .astype(ml_dtypes.bfloat16))


def _emit(tc, aps, widths):
    """Emit the per-core program. widths: 4 per-slot query widths (64k)."""
    from collections import deque
    from contextlib import ExitStack

    import concourse.mybir as mybir

    nc = tc.nc
    f32 = mybir.dt.float32
    bf16 = mybir.dt.bfloat16
    EXP = mybir.ActivationFunctionType.Exp
    COPY = mybir.ActivationFunctionType.Copy
    LN = mybir.ActivationFunctionType.Ln

    xT, wqT, wkT, wvT, woT, mask, out = (
        aps["xT"], aps["wqT"], aps["wkT"], aps["wvT"], aps["woT"],
        aps["mask"], aps["out"],
    )
    nchunks = [(w + 511) // 512 for w in widths]  # computed chunks per slot

    def swidth(slot, i4):
        """Width of slot's chunk i4 (<=512, multiple of 64)."""
        return min(512, widths[slot] - 512 * i4)

    pw = [max(widths[0], widths[1]), max(widths[2], widths[3])]
    pchunks = [(w + 511) // 512 for w in pw]      # q chunks per pair

    ctx = ExitStack()
    with ctx:
        sb = ctx.enter_context(tc.tile_pool(name="sb", bufs=1))
        # PSUM banks: scores 2x2 + projections 2x1 + attn@V accum 2x1 = 8
        ps_s = ctx.enter_context(tc.tile_pool(name="ps_s", bufs=2,
                                              space="PSUM"))
        ps_p = ctx.enter_context(tc.tile_pool(name="ps_p", bufs=2,
                                              space="PSUM"))
        psav = ctx.enter_context(tc.tile_pool(name="psav", bufs=2,
                                              space="PSUM"))
        rot = ctx.enter_context(tc.tile_pool(name="rot", bufs=EX_BUFS))
        ost = ctx.enter_context(tc.tile_pool(name="ost", bufs=3))
        sml = ctx.enter_context(tc.tile_pool(name="sml", bufs=4))
        xw = ctx.enter_context(tc.tile_pool(name="xw", bufs=1))
        qd_pool = ctx.enter_context(tc.tile_pool(name="qd", bufs=1))

        # persistent intermediates
        q_sb = [sb.tile([P, S], bf16, name=f"q{p}") for p in range(2)]
        k_sb = [sb.tile([P, S], bf16, name=f"k{p}") for p in range(2)]
        v_sb = [sb.tile([P, HPG, DH + 1], bf16, name=f"v{t}")
                for t in range(16)]
        a_sb = [sb.tile([P, S], bf16, name=f"a{c}") for c in range(2)]
        meanv = sb.tile([64, HPG], bf16, name="meanv")
        ones_sb = sb.tile([1, 64], bf16, name="ones")
        # per-head k with the head's 64 dims duplicated into both partition
        # halves: lets two key-tiles' score matmuls run CONCURRENTLY on
        # disjoint PE row-groups (tile_position packing)
        khd = [sb.tile([P, S], bf16, name=f"khd{h}") for h in range(HPG)]

        # ---- input loads: spread across the three DMA-capable queues.
        # x / mask / wo are CHUNKED column-wise into separate tiles so the
        # 512KB loads split into parallel 128KB transfers on distinct
        # hardware DMA queues AND the first k/q projections start after
        # only chunk 0 has landed (a single 512KB transfer occupies one
        # ~21GB/s queue for ~24us -- that was the entire ramp).
        # input loads split across the sync+scalar rings (both fan out
        # across the parallel hardware DMA queues; gpsimd-ring DMAs
        # execute as ~630ns serial DIRECT2D copies on the Pool engine and
        # would gate the ramp). The gpsimd ring carries only the khd/qd
        # duplication copies, which are small and latency-critical.
        engs = [nc.sync, nc.scalar]
        x_sb = [[xw.tile([P, 512], bf16, name=f"x{d}_{c}")
                 for c in range(NCH)] for d in range(8)]
        wq_sb = [xw.tile([P, GW], bf16, name=f"wq{d}") for d in range(8)]
        wk_sb = [xw.tile([P, GW], bf16, name=f"wk{d}") for d in range(8)]
        wv_sb = [xw.tile([P, GW], bf16, name=f"wv{d}") for d in range(8)]
        mk_sb = [[xw.tile([P, 512], bf16, name=f"mk{p}_{c}")
                  for c in range(NCH)] for p in range(2)]
        wo_sb2 = [[xw.tile([P, 512], bf16, name=f"wo{c}_{n}")
                   for n in range(2)] for c in range(2)]
        # warm the ACT exp table-set (~2.7us load) during the DMA phase
        # so the first real exp doesn't pay it on the critical path
        warm_in = sml.tile([1, 8], f32, name="warm_in")
        warm_out = sml.tile([1, 8], f32, name="warm_out")
        nc.any.memset(warm_in[:], 0.0)
        nc.scalar.activation(warm_out[:], warm_in[:], EXP,
                             bias=0.0, scale=0.125)
        nc.any.memset(ones_sb[:], 1.0)

        # prologue loads ONLY what the first k/q chunk needs; every
        # later input is a deadline filler so the engine rings stay short
        # (a dma_start waits behind all earlier ones on its ring)
        dma_n = [0]

        def dma(dst, src_ap):
            engs[dma_n[0] % 2].dma_start(dst, src_ap)
            dma_n[0] += 1

        for d in range(8):
            dma(wk_sb[d][:], wkT[d * P:(d + 1) * P, :])
        for d in range(8):
            dma(wq_sb[d][:], wqT[d * P:(d + 1) * P, :])
        for d in range(8):
            dma(x_sb[d][0][:], xT[0, d * P:(d + 1) * P, :])
        dma(mk_sb[0][0][:], mask[0, 0])

        def load_x_chunk(c):
            for d in range(8):
                dma(x_sb[d][c][:], xT[c, d * P:(d + 1) * P, :])
            dma(mk_sb[0][c][:], mask[0, c])

        def load_wv():
            for d in range(8):
                dma(wv_sb[d][:], wvT[d * P:(d + 1) * P, :])

        def load_mask1():
            for c in range(NCH):
                dma(mk_sb[1][c][:], mask[1, c])

        def load_wo():
            for c in range(2):
                for n2 in range(2):
                    dma(wo_sb2[c][n2][:], woT[c, n2, :, :])

        # ---- emitters ----------------------------------------------------
        k_pt = {}

        def emit_k_half(mt, n4, half):
            if half == 0:
                k_pt[(mt, n4)] = ps_p.tile([P, 512], f32, name="ps_p")
            pt = k_pt[(mt, n4)]
            for d in range(4 * half, 4 * half + 4):
                nc.tensor.matmul(
                    pt[:],
                    wk_sb[d][:, mt * P:(mt + 1) * P],
                    x_sb[d][n4][:],
                    start=(d == 0), stop=(d == 7),
                )
            if half == 0:
                return
            del k_pt[(mt, n4)]
            nc.vector.tensor_copy(
                k_sb[mt][:, n4 * 512:(n4 + 1) * 512], pt[:])
            cs = slice(n4 * 512, (n4 + 1) * 512)
            for rr in range(2):
                h = 2 * mt + rr
                src_ap = k_sb[mt][64 * rr:64 * rr + 64, cs]
                nc.gpsimd.dma_start(khd[h][0:64, cs], src_ap)
                nc.gpsimd.dma_start(khd[h][64:128, cs], src_ap)

        def emit_k_chunk(mt, n4):
            emit_k_half(mt, n4, 0)
            emit_k_half(mt, n4, 1)

        q_pt = {}

        def emit_q_half(mt, n4, half):
            qw = min(512, pw[mt] - 512 * n4)
            if half == 0:
                q_pt[(mt, n4)] = ps_p.tile([P, 512], f32, name="ps_p")
            pt = q_pt[(mt, n4)]
            for d in range(4 * half, 4 * half + 4):
                nc.tensor.matmul(
                    pt[:, :qw],
                    wq_sb[d][:, mt * P:(mt + 1) * P],
                    x_sb[d][n4][:, :qw],
                    start=(d == 0), stop=(d == 7),
                )
            if half == 0:
                return
            del q_pt[(mt, n4)]
            cs = slice(n4 * 512, n4 * 512 + qw)
            # fold the row mask into q (masked query -> q = 0)
            nc.vector.tensor_mul(
                q_sb[mt][:, cs], pt[:, :qw], mk_sb[mt][n4][:, :qw])
            for rr in range(2):
                if n4 >= nchunks[2 * mt + rr]:
                    continue
                sw = swidth(2 * mt + rr, n4)
                qd = qd_pool.tile([P, 512], bf16, name=f"qd{mt}_{n4}_{rr}")
                qd_tiles[(mt, n4, rr)] = qd
                src_ap = q_sb[mt][64 * rr:64 * rr + 64,
                                  n4 * 512:n4 * 512 + sw]
                nc.gpsimd.dma_start(qd[0:64, :sw], src_ap)
                nc.gpsimd.dma_start(qd[64:128, :sw], src_ap)

        def emit_q_chunk(mt, n4):
            emit_q_half(mt, n4, 0)
            emit_q_half(mt, n4, 1)

        def emit_v_tile(t):
            pt = ps_p.tile([P, 512], f32, name="ps_p")[:, :GW]
            for d in range(8):
                nc.tensor.matmul(
                    pt,
                    x_sb[d][t // 4][:, (t % 4) * P:(t % 4 + 1) * P],
                    wv_sb[d][:],
                    start=(d == 0), stop=(d == 7),
                )
            nc.any.memset(v_sb[t][:], 1.0)   # ones column at [:, :, DH]
            nc.vector.tensor_copy(
                v_sb[t][:, :, 0:DH],
                pt.rearrange("p (h e) -> p h e", h=HPG))

        def emit_meanv():
            # mean of V per head (= output of fully-masked query rows)
            pmv = ps_p.tile([P, 512], f32, name="ps_p")[:DH + 1, :HPG]
            for h in range(HPG):
                for jt in range(16):
                    nc.tensor.matmul(
                        pmv[:, h:h + 1],
                        v_sb[jt][:, h, :],
                        v_sb[jt][:, h, DH:DH + 1],  # the ones column
                        start=(jt == 0), stop=(jt == 15),
                    )
            nc.scalar.activation(meanv[:], pmv[:DH, :], COPY,
                                 bias=0.0, scale=1.0 / S)

        def emit_fills():
            # masked query cols: attention output is exactly mean-of-V
            for pair in range(2):
                for rr in range(2):
                    h = 2 * pair + rr
                    rows = slice(64 * rr, 64 * rr + 64)
                    if widths[h] < S:
                        nc.vector.tensor_copy(
                            a_sb[pair][rows, widths[h]:S],
                            meanv[:, h:h + 1].to_broadcast(
                                (64, S - widths[h])),
                        )

        def emit_final_unit(i4, t4, tail):
            """Output projection for one token tile."""
            t = i4 * 4 + t4
            ot = ost.tile([P, D], f32, name="ot")
            for n2 in range(2):
                pf = ps_p.tile([P, 512], f32, name="ps_p")
                for c in range(2):
                    nc.tensor.matmul(
                        pf[:],
                        a_sb[c][:, t * P:(t + 1) * P],
                        wo_sb2[c][n2][:],
                        start=(c == 0), stop=(c == 1),
                    )
                if tail and n2 == 1:
                    # exp stream is drying up: borrow the ACT for half
                    # the copies so the ps_p rotation keeps pace
                    nc.scalar.activation(
                        ot[:, n2 * 512:(n2 + 1) * 512], pf[:], COPY)
                else:
                    nc.vector.tensor_copy(
                        ot[:, n2 * 512:(n2 + 1) * 512], pf[:])
            (nc.scalar if t % 2 else nc.sync).dma_start(
                out[t * P:(t + 1) * P, :], ot[:])

        class Site:
            """One (chunk, pair, head-row) attention block."""

            def __init__(self, i4, pair, rr):
                self.i4, self.pair, self.rr = i4, pair, rr
                self.h = 2 * pair + rr
                self.W = swidth(self.h, i4)
                self.rows = slice(64 * rr, 64 * rr + 64)
                self.qs = slice(i4 * 512, i4 * 512 + self.W)
                self.pav = psav.tile([DH + 1, 512], f32, name="psav")
                self.pses = []
                self.exs = []

            def emit_scores(self, jtp):
                W = self.W
                pse = ps_s.tile([P, 2, 512], f32, name="ps_s")
                # the two key-tiles use disjoint PE row-groups (partitions
                # 0-63 / 64-127 of the duplicated khd/qd tiles), so they
                # execute concurrently (the 2nd matmul costs ~3ns)
                qd = qd_tiles[(self.pair, self.i4, self.rr)]
                for jj in range(2):
                    jt = jtp * 2 + jj
                    half = slice(64 * jj, 64 * jj + 64)
                    # scores^T = k @ q^T for head h
                    nc.tensor.matmul(
                        pse[:, jj, :W],
                        khd[self.h][half, jt * P:(jt + 1) * P],
                        qd[half, :W],
                        start=True, stop=True,
                    )
                self.pses.append(pse)

            def emit_exp(self, jtp):
                W = self.W
                ex = rot.tile([P, 2, 512], bf16, name="ex")
                nc.scalar.activation(ex[:, :, :W], self.pses[jtp][:, :, :W],
                                     EXP, bias=0.0, scale=0.125)
                self.exs.append(ex)

            def emit_av(self, jtp):
                W = self.W
                ex = self.exs[jtp]
                for jj in range(2):
                    jt = jtp * 2 + jj
                    nc.tensor.matmul(
                        self.pav[:, :W],
                        v_sb[jt][:, self.h, :],
                        ex[:, jj, :W],
                        start=(jtp == 0 and jj == 0),
                        stop=(jtp == 7 and jj == 1),
                    )

            def emit_norm_a(self):
                # DMA-free normalization, phase A (DVE): exact reciprocal()
                # on one partition costs 3.3us (it iterates per element on
                # a single lane); the approx custom op (~51 ULP, far below
                # the bf16 rounding below) is ~5x faster. It reads the
                # denominator via a staging copy: fed straight from the
                # psum row it returns garbage (microtested fine from
                # SBUF/PSUM at offset 0, so it is the offset-64 psum row
                # of an accumulating tile it mishandles).
                W = self.W
                self.rcp = sml.tile([1, 512], bf16, name="rcp")
                dn = sml.tile([1, 512], f32, name="dn")
                nc.vector.tensor_copy(dn[:, :W], self.pav[DH:DH + 1, :W])
                r32 = sml.tile([1, 512], f32, name="r32")
                nc.vector.reciprocal_approx_fast(
                    out=r32[:, :W], in_=dn[:, :W])
                nc.vector.tensor_copy(self.rcp[:, :W], r32[:, :W])

            def emit_norm_b(self):
                # phase B (two steps later, so the PE never waits on phase
                # A): broadcast across 64 partitions via a ones[1,64]
                # stationary matmul, bounce to SBUF (exact: values are
                # already bf16-rounded; DVE reads only one PSUM operand),
                # multiply into a_sb.
                W = self.W
                rcb = ps_p.tile([P, 512], f32, name="ps_p")
                nc.tensor.matmul(rcb[:64, :W], ones_sb[:],
                                 self.rcp[:, :W], start=True, stop=True)
                rcs = sml.tile([64, 512], bf16, name="rcs")
                nc.vector.tensor_copy(rcs[:, :W], rcb[:64, :W])
                nc.vector.tensor_mul(
                    a_sb[self.pair][self.rows, self.qs],
                    self.pav[0:DH, :W], rcs[:, :W])

        # ---- stream schedule --------------------------------------------
        # pair 0 leads, pair 1 lags one chunk
        site_list = []
        for i4 in range(NCH + 1):
            if i4 < NCH:
                for rr in range(2):
                    if i4 < nchunks[rr]:
                        site_list.append((i4, 0, rr))
            if 1 <= i4:
                for rr in range(2):
                    if i4 - 1 < nchunks[2 + rr]:
                        site_list.append((i4 - 1, 1, rr))

        first_step = {}        # (i4, pair) -> first step of any matching site
        for j, (i4, pair, rr) in enumerate(site_list):
            first_step.setdefault((i4, pair), 8 * j)
        fs1 = min((8 * j for j, s in enumerate(site_list) if s[1] == 1),
                  default=10**9)

        qd_tiles = {}

        # deadline-scheduled fillers: (due_step, order, fn)
        fillers = []
        fillers.append((0, -1, lambda: load_x_chunk(1)))
        fillers.append((1, -1, load_wv))
        fillers.append((2, -1, lambda: load_x_chunk(2)))
        fillers.append((4, -1, lambda: load_x_chunk(3)))
        # mask[1] MUST be emitted before the first q(pair1) filler: a
        # read emitted before its producing DMA gets no dependency at all
        q1_due = max(1, first_step[(0, 1)] - 10)
        fillers.append((max(1, q1_due - 2), -1, load_mask1))
        fillers.append((24, -1, load_wo))
        for c in range(1, NCH):
            for half in range(2):
                fillers.append((2 * c - 2 + half, 0,
                                lambda c=c, h=half: emit_k_half(0, c, h)))
        for c in range(NCH):
            for half in range(2):
                fillers.append((max(8, fs1 - 14 + 3 * c + half), 1,
                                lambda c=c, h=half: emit_k_half(1, c, h)))
        for pair in range(2):
            for n4 in range(pchunks[pair]):
                if (pair, n4) == (0, 0):
                    continue
                due = max(1, first_step[(n4, pair)] - 10)
                for half in range(2):
                    fillers.append((due + half, 2,
                                    lambda p=pair, n=n4, h=half:
                                    emit_q_half(p, n, h)))
        for t in range(16):
            fillers.append((4 + t, 3, lambda t=t: emit_v_tile(t)))
        fillers.append((21, 4, emit_meanv))
        fillers.append((21, 5, emit_fills))
        fillers.sort(key=lambda f: (f[0], f[1]))
        fptr = [0]

        def do_fillers(step):
            while fptr[0] < len(fillers) and fillers[fptr[0]][0] <= step:
                fillers[fptr[0]][2]()
                fptr[0] += 1

        norms_left = [0] * NCH
        for (i4, pair, rr) in site_list:
            norms_left[i4] += 1
        pending_finals = deque()
        n_units = [0]
        total_units = 4 * NCH

        def queue_final(c):
            for t4 in range(4):
                pending_finals.append((c, t4))

        def emit_one_final():
            if pending_finals:
                c, t4 = pending_finals.popleft()
                n_units[0] += 1
                emit_final_unit(c, t4, tail=(n_units[0] > total_units - 8))

        normb_q = deque()     # (due_step, site)

        def drain(sj, step):
            site, jtp = sj
            site.emit_av(jtp)
            if jtp == 7:
                site.emit_norm_a()
                normb_q.append((step + 2, site))

        def do_normb(step):
            while normb_q and normb_q[0][0] <= step:
                site = normb_q.popleft()[1]
                site.emit_norm_b()
                norms_left[site.i4] -= 1
                if norms_left[site.i4] == 0:
                    queue_final(site.i4)

        # prologue
        emit_k_chunk(0, 0)
        emit_q_chunk(0, 0)

        avq = deque()
        step = 0
        for key in site_list:
            site = Site(*key)
            for jtp in range(8):
                do_fillers(step)
                do_normb(step)
                site.emit_scores(jtp)
                site.emit_exp(jtp)
                avq.append((site, jtp))
                if len(avq) > AV_LAG:
                    drain(avq.popleft(), step)
                emit_one_final()
                step += 1
        while avq or normb_q:
            do_fillers(step)
            do_normb(step)
            if avq:
                drain(avq.popleft(), step)
            emit_one_final()
            step += 1
        do_fillers(10**9)
        while pending_finals:
            emit_one_final()


def build_program(widths):
    """Build + schedule + compile the per-core program (cached per key)."""
    widths = tuple(widths)
    if widths in _PROGS:
        return _PROGS[widths]

    import concourse.mybir as mybir
    import concourse.tile as tile
    from concourse import bacc

    nc = bacc.Bacc("TRN2", target_bir_lowering=False, debug=False)
    f32 = mybir.dt.float32
    bf16 = mybir.dt.bfloat16
    aps = {
        "xT": nc.dram_tensor("xT", [NCH, D, 512], bf16,
                             kind="ExternalInput").ap(),
        "wqT": nc.dram_tensor("wqT", [D, GW], bf16, kind="ExternalInput").ap(),
        "wkT": nc.dram_tensor("wkT", [D, GW], bf16, kind="ExternalInput").ap(),
        "wvT": nc.dram_tensor("wvT", [D, GW], bf16, kind="ExternalInput").ap(),
        "woT": nc.dram_tensor("woT", [2, 2, P, 512], bf16,
                              kind="ExternalInput").ap(),
        "mask": nc.dram_tensor("mask", [2, NCH, P, 512], bf16,
                               kind="ExternalInput").ap(),
        "out": nc.dram_tensor("out", [S, D], f32, kind="ExternalOutput").ap(),
    }
    with tile.TileContext(nc) as tc:
        _emit(tc, aps, widths)
    nc.compile()
    _PROGS[widths] = nc
    return nc


def plan(valid_lens):
    """Head->core assignment and the compile-time width tuple.

    Returns (widths, heads_per_core): heads_per_core[c] lists the 4
    global head indices (within core c's batch) in slot order.
    """
    valid = np.asarray(valid_lens).reshape(B, H)
    heads_per_core = [None] * N_CORES
    quart_max = [0] * HPG
    for b in range(B):
        order = np.argsort(-valid[b], kind="stable")
        for j in range(HPG):
            hs = [int(order[4 * i + j]) for i in range(HPG)]
            heads_per_core[b * HPG + j] = hs
        for i in range(HPG):
            quart_max[i] = max(quart_max[i],
                               int(valid[b, order[4 * i]]))
    widths = tuple(min(-(-m // 64) * 64, S) for m in quart_max)
    return widths, heads_per_core


def make_in_maps(X, Wq, Wk, Wv, Wo, valid_lens):
    """Host-side sharding: build the 8 per-core input maps."""
    import ml_dtypes
    X = np.asarray(X, dtype=np.float32)
    valid = np.asarray(valid_lens).reshape(B, H)
    widths, heads_per_core = plan(valid_lens)
    iota = np.arange(S)
    in_maps = []
    # x chunked [NCH, D, 512] so each 128KB slab is one contiguous DMA
    xTs = [np.ascontiguousarray(
        _to_bf16(X[b].T).reshape(D, NCH, 512).transpose(1, 0, 2))
        for b in range(B)]
    Wq, Wk, Wv, Wo = (np.asarray(a, np.float32) for a in (Wq, Wk, Wv, Wo))
    for c in range(N_CORES):
        b = c // HPG
        hs = heads_per_core[c]
        rows = np.concatenate([np.arange(h * DH, (h + 1) * DH) for h in hs])
        mask = np.empty((2, P, S), dtype=ml_dtypes.bfloat16)
        for p in range(2):
            for rr in range(2):
                h = hs[2 * p + rr]
                mask[p, 64 * rr:64 * rr + 64, :] = (
                    iota < int(valid[b, h])).astype(ml_dtypes.bfloat16)[None, :]
        mask_c = np.ascontiguousarray(
            mask.reshape(2, P, NCH, 512).transpose(0, 2, 1, 3))
        woT = _to_bf16(Wo[:, rows].T)          # [GW, D]
        woT_c = np.ascontiguousarray(
            woT.reshape(2, P, 2, 512).transpose(0, 2, 1, 3))
        in_maps.append({
            "xT": xTs[b],
            "wqT": _to_bf16(Wq[rows, :].T),
            "wkT": _to_bf16(Wk[rows, :].T),
            "wvT": _to_bf16(Wv[rows, :].T),
            "woT": woT_c,
            "mask": mask_c,
        })
    return widths, in_maps


def assemble(results, Wo, bv, bo):
    """Host-side unshard: sum row-parallel partials, fold bv/bo exactly."""
    out = np.zeros((B, S, D), dtype=np.float32)
    for c in range(N_CORES):
        b = c // HPG
        out[b] += results[c]["out"]
    bias = (np.asarray(bv, np.float32) @ np.asarray(Wo, np.float32).T
            + np.asarray(bo, np.float32))
    out += bias[None, None, :]
    return out


def _numpy_fallback(X, Wq, bq, Wk, bk, Wv, bv, Wo, bo, valid_lens):
    X = np.asarray(X, np.float32)
    q = (X @ np.asarray(Wq, np.float32).T + np.asarray(bq, np.float32))
    k = (X @ np.asarray(Wk, np.float32).T + np.asarray(bk, np.float32))
    v = (X @ np.asarray(Wv, np.float32).T + np.asarray(bv, np.float32))

    def split(y):
        return (y.reshape(B, S, H, DH).transpose(0, 2, 1, 3)
                .reshape(B * H, S, DH))

    q, k, v = split(q), split(k), split(v)
    s = np.einsum("bqd,bkd->bqk", q, k) / np.sqrt(DH).astype(np.float32)
    rm = (np.arange(S)[None, :]
          < np.asarray(valid_lens).reshape(-1)[:, None])
    s = np.where(rm[:, :, None], s, -1e6)
    s = s - s.max(axis=-1, keepdims=True)
    e = np.exp(s)
    attn = e / e.sum(axis=-1, keepdims=True)
    o = np.einsum("bqk,bkd->bqd", attn, v)
    o = o.reshape(B, H, S, DH).transpose(0, 2, 1, 3).reshape(B, S, D)
    return o @ np.asarray(Wo, np.float32).T + np.asarray(bo, np.float32)


def run_cores(widths, in_maps, trace=False, **kw):
    """Run the compiled program on cores 0-7."""
    from concourse.bass_utils import run_bass_kernel_spmd

    nc = build_program(widths)
    return run_bass_kernel_spmd(nc, in_maps, list(range(N_CORES)),
                                trace=trace, **kw)


def kernel(X, Wq, bq, Wk, bk, Wv, bv, Wo, bo, valid_lens):
    if np.any(np.asarray(bq)) or np.any(np.asarray(bk)):
        # never the case for this problem's setup_inputs (zeros);
        # exact fallback kept for safety.
        return _numpy_fallback(X, Wq, bq, Wk, bk, Wv, bv, Wo, bo, valid_lens)
    widths, in_maps = make_in_maps(X, Wq, Wk, Wv, Wo, valid_lens)
    res = run_cores(widths, in_maps, trace=False)
    return assemble(res.results, Wo, bv, bo)
